# revision 1
# baseline (speedup 1.0000x reference)
"""HMM log-likelihood (log-domain forward algorithm) on 8 Trainium2 cores.

Strategy: scaled linear-domain forward algorithm with warmup-halo sequence
parallelism.  The filtering distribution of an HMM forgets its initial
condition geometrically fast, so N=1e6 timesteps are split into 3840
independent chains (480/core); each chain starts from a uniform state W=20
steps before its owned region of L=260 steps.  Per core, chains are batched
4-wide across the 128 SBUF partitions (block-diagonal T^T weights on the PE)
with the chain-block index in the matmul free dimension, so each timestep is
one bf16 matmul (T @ S into PSUM) plus one vector multiply by the emission
probabilities.

Normalization is free: a constant per-step drift delta = E[log c] is folded
into the exp bias, making log|S| a zero-drift random walk (~26 bits 4.5
sigma over a 280-step chain — far inside f32 range), so the kernel needs no
per-chain rescaling.  The bf16 quantization of T factors exactly as
D_r @ T_hat with T_hat row-stochastic; -log(r) is folded into the same exp
bias.  Each chain's contribution is log(sum(S_final)) - log(sum(S_at_W)) +
delta*L, assembled on the host, which also runs exact f64 scans for the
prefix [0, W) and the short tail.
"""

import sys

for p in ("/opt/trn_rl_repo", "/root/.axon_site", "/root/.axon_site/_ro/trn_rl_repo",
          "/root/.axon_site/_ro/pypackages"):
    if p not in sys.path:
        sys.path.insert(0, p)

import numpy as np

K = 32
N = 1_000_000
NCORES = 8
W = 20            # warmup (halo) steps per chain
L = 260           # owned steps per chain
CC = 480          # chains per core
SPAN = W + L      # 280 sequential steps
SBLK = 140        # timesteps per load window
NWIN = SPAN // SBLK
NB = CC // 4      # 120 four-chain blocks
G = 2             # interleaved compute groups
F = NB // G       # 60 blocks (matmul free dim) per group
NSL = CC * L + W  # per-core input slice columns
COVERED = W + NCORES * CC * L

_cache = {}


def _build():
    import concourse.bass as bass
    import concourse.bacc as bacc
    import concourse.mybir as mybir
    import concourse.tile as tile
    from contextlib import ExitStack

    f32 = mybir.dt.float32
    bf16 = mybir.dt.bfloat16
    AF = mybir.ActivationFunctionType

    nc = bacc.Bacc("TRN2", target_bir_lowering=False, debug=False,
                   num_devices=NCORES)
    x = nc.dram_tensor("x", [K, NSL], f32, kind="ExternalInput")
    wmat = nc.dram_tensor("wmat", [128, 128], bf16, kind="ExternalInput")
    ebias = nc.dram_tensor("ebias", [128, 1], f32, kind="ExternalInput")
    snap_out = nc.dram_tensor("snap_out", [128, NB], bf16, kind="ExternalOutput")
    fin_out = nc.dram_tensor("fin_out", [128, NB], bf16, kind="ExternalOutput")

    with tile.TileContext(nc) as tc:
        with ExitStack() as ctx:
            cpool = ctx.enter_context(tc.tile_pool(name="const", bufs=1))
            rpool = ctx.enter_context(tc.tile_pool(name="rp", bufs=NWIN))
            pspool = ctx.enter_context(
                tc.tile_pool(name="ps", bufs=2, space=bass.MemorySpace.PSUM))

            w_t = cpool.tile([128, 128], bf16, tag="w")
            nc.sync.dma_start(w_t[:], wmat[:])
            eb_t = cpool.tile([128, 1], f32, tag="eb")
            nc.sync.dma_start(eb_t[:], ebias[:])

            spool = ctx.enter_context(tc.tile_pool(name="sp", bufs=2))
            S, SN = [], []
            for g in range(G):
                st = spool.tile([128, F], bf16, tag=f"S{g}", name=f"st{g}")
                nc.vector.memset(st[:], 1.0)
                sn = cpool.tile([128, F], bf16, tag=f"N{g}")
                S.append(st)
                SN.append(sn)

            # Load + exp windows.  R[g][w] layout: [128, F, SBLK], partition
            # p = 32*q + k holds chain (g*F + cb)*4 + q, state k.
            R = [[None] * NWIN for _ in range(G)]
            NCHUNK = 4
            CH = F // NCHUNK
            # interleave DMA chunks and exp chunks across groups so both
            # chains become runnable at the same (early) time
            for w in range(NWIN):
                for g in range(G):
                    rt = rpool.tile([128, F, SBLK], f32, tag=f"R{g}",
                                    name=f"rt{g}_{w}")
                    R[g][w] = rt
                for ch in range(NCHUNK):
                    for g in range(G):
                        rt = R[g][w]
                        cb0 = ch * CH
                        for q in range(4):
                            off = ((g * F + cb0) * 4 + q) * L + w * SBLK
                            src = bass.AP(x, off,
                                          [[NSL, 32], [4 * L, CH], [1, SBLK]])
                            nc.sync.dma_start(
                                rt[32 * q:32 * q + 32, cb0:cb0 + CH, :], src)
                # exp in place, chunked along s so compute starts early
                EC = 7
                for ec in range(EC):
                    for g in range(G):
                        rt = R[g][w]
                        s0 = ec * (SBLK // EC)
                        nc.scalar.activation(
                            rt[:, :, s0:s0 + SBLK // EC],
                            rt[:, :, s0:s0 + SBLK // EC], AF.Exp,
                            bias=eb_t[:])

            for s in range(SPAN):
                w, si = divmod(s, SBLK)
                for g in range(G):
                    ps = pspool.tile([128, F], f32, tag=f"mm{g}")
                    nc.tensor.matmul(ps[:], w_t[:], S[g][:], start=True, stop=True)
                    # ping-pong the state tile so the multiply never WARs
                    # against this step's matmul read
                    sn_new = spool.tile([128, F], bf16, tag=f"S{g}",
                                        name=f"st{g}_{s}")
                    nc.vector.tensor_mul(sn_new[:], ps[:], R[g][w][:, :, si])
                    S[g] = sn_new
                    if s == W - 1:
                        nc.vector.tensor_copy(SN[g][:], S[g][:])

            for g in range(G):
                nc.sync.dma_start(snap_out[:, g * F:(g + 1) * F], SN[g][:])
                nc.sync.dma_start(fin_out[:, g * F:(g + 1) * F], S[g][:])

    nc.compile()
    return nc


def _get_nc():
    if "nc" not in _cache:
        _cache["nc"] = _build()
    return _cache["nc"]


def _log_softmax64(v, axis):
    v = v.astype(np.float64)
    m = v.max(axis=axis, keepdims=True)
    e = np.exp(v - m)
    return v - m - np.log(e.sum(axis=axis, keepdims=True))


def _estimate_delta(log_pdf, T64):
    # E[log c] from a vectorized short scan: 64 parallel probes, 56 steps,
    # burn-in 16 (mixing time is ~10 steps).
    NCH, NST, BURN = 64, 56, 16
    cols = np.arange(NCH) * 997 + 1
    a = np.full((K, NCH), 1.0 / K)
    samples = []
    for s in range(NST):
        p = np.exp(log_pdf[:, cols + s].astype(np.float64))
        a = p * (T64 @ a)
        c = a.sum(axis=0)
        a /= c
        if s >= BURN:
            samples.append(np.log(c))
    return float(np.mean(samples))


def _make_in_maps(log_pdf, T64):
    from ml_dtypes import bfloat16

    T32 = T64.astype(np.float32)
    Tbf = T32.astype(bfloat16)
    delta = _estimate_delta(log_pdf, T64)
    # bf16-quantized T is exactly D_r @ T_hat with T_hat row-stochastic and
    # r the bf16 row sums; fold -log(r) and the drift -delta into the exp.
    r = Tbf.astype(np.float64).sum(axis=1)
    eb = np.zeros((128, 1), dtype=np.float32)
    for q in range(4):
        eb[32 * q:32 * q + 32, 0] = (-np.log(r) - delta).astype(np.float32)
    wm = np.zeros((128, 128), dtype=bfloat16)
    for q in range(4):
        wm[32 * q:32 * q + 32, 32 * q:32 * q + 32] = Tbf.T
    in_maps = []
    for k in range(NCORES):
        c0 = k * CC * L
        in_maps.append({
            "x": np.ascontiguousarray(log_pdf[:, c0:c0 + NSL]),
            "wmat": wm,
            "ebias": eb,
        })

    return in_maps, delta


def kernel(log_pdf: np.ndarray, pi: np.ndarray, T: np.ndarray) -> np.ndarray:
    from concourse.bass_utils import run_bass_kernel_spmd

    log_pdf = np.ascontiguousarray(log_pdf, dtype=np.float32)
    log_pi64 = _log_softmax64(pi, 0)
    log_T64 = _log_softmax64(T, 1)
    T64 = np.exp(log_T64)                     # row-stochastic [K, K] f64

    in_maps, delta = _make_in_maps(log_pdf, T64)
    nc = _get_nc()
    res = run_bass_kernel_spmd(nc, in_maps, list(range(NCORES))).results

    # ---- host combine (f64) ----
    LP = log_pdf
    # exact prefix [0, W)
    a = np.exp(log_pi64 + LP[:, 0].astype(np.float64))
    c = a.sum()
    total = np.log(c)
    a /= c
    for t in range(1, W):
        a = np.exp(LP[:, t].astype(np.float64)) * (T64 @ a)
        c = a.sum()
        total += np.log(c)
        a /= c

    # per-chain contributions: log(sum fin) - log(sum snap) + delta*L
    for k in range(NCORES):
        snap = res[k]["snap_out"].astype(np.float64)   # [128, NB]
        fin = res[k]["fin_out"].astype(np.float64)
        for q in range(4):
            ssum = snap[32 * q:32 * q + 32, :].sum(axis=0)
            fsum = fin[32 * q:32 * q + 32, :].sum(axis=0)
            total += (np.log(fsum) - np.log(ssum)).sum() + delta * L * NB

    # exact tail [COVERED, N) from the last chain's final state
    k, g, cb, q = NCORES - 1, G - 1, F - 1, 3
    fv = res[k]["fin_out"][32 * q:32 * q + 32, g * F + cb].astype(np.float64)
    a = fv / fv.sum()
    for t in range(COVERED, N):
        a = np.exp(LP[:, t].astype(np.float64)) * (T64 @ a)
        c = a.sum()
        total += np.log(c)
        a /= c

    return np.float32(total)



# revision 2
# speedup vs baseline: 3.5590x; 3.5590x over previous
"""HMM log-likelihood (log-domain forward algorithm) on 8 Trainium2 cores.

Scaled linear-domain forward algorithm with warmup-halo sequence
parallelism.  N=1e6 timesteps are split into 24960 independent chains
(3120/core); each chain starts from a uniform state W=6 steps before its
owned region of L=40 steps (the HMM mixes with |lambda2|~0.24, so 6
warmup steps reach the bf16 noise floor).  Per core, chains are batched
4-wide across the 128 SBUF partitions (block-diagonal T^T weights on the
PE) with the chain-block index in the matmul free dimension, G=2
interleaved groups of F=390 blocks, so each timestep per group is one
bf16 matmul (T @ S into PSUM) plus one vector multiply by the emission
probabilities.

The emissions exp(log_pdf - delta - log r) are computed on the host in
f32, quantized to bf16, and repacked into the exact per-step SBUF layout
[128, SPAN*NB], so the device does no exp and the DMA is a handful of
large contiguous window loads.  delta = E[log c] makes log|S| a
zero-drift random walk; the bf16 quantization of T factors exactly as
D_r @ T_hat with T_hat row-stochastic, and -log(r) is folded into the
same host-side exponent.  All matmuls share one stationary weight load
(ldweights=False on all but the first per group).  Each chain's
contribution is log(sum(S_final)) - log(sum(S_at_W)) + delta*L,
assembled on the host, which also runs exact f64 scans for the prefix
[0, W) and the short tail.
"""

import sys

for p in ("/opt/trn_rl_repo", "/root/.axon_site", "/root/.axon_site/_ro/trn_rl_repo",
          "/root/.axon_site/_ro/pypackages"):
    if p not in sys.path:
        sys.path.insert(0, p)

import numpy as np

K = 32
N = 1_000_000
NCORES = 8
W = 6             # warmup (halo) steps per chain
L = 40            # owned steps per chain
SPAN = W + L      # 46 sequential steps
CC = 124800 // L  # 3120 chains per core
NB = CC // 4      # 780 four-chain blocks
G = 2             # interleaved compute groups
F = NB // G       # 390 blocks (matmul free dim) per group
TOT = SPAN * NB   # input columns per core
COVERED = W + NCORES * CC * L

# window sizes in steps (first small for fast ramp)
WIN_STEPS = [2] + [4] * 11
assert sum(WIN_STEPS) == SPAN

_cache = {}


def _build():
    import concourse.bass as bass
    import concourse.bacc as bacc
    import concourse.mybir as mybir
    import concourse.tile as tile
    from contextlib import ExitStack

    f32 = mybir.dt.float32
    bf16 = mybir.dt.bfloat16

    nc = bacc.Bacc("TRN2", target_bir_lowering=False, debug=False,
                   num_devices=NCORES)
    x = nc.dram_tensor("x", [128, TOT], bf16, kind="ExternalInput")
    wmat = nc.dram_tensor("wmat", [128, 128], bf16, kind="ExternalInput")
    snap_out = nc.dram_tensor("snap_out", [128, NB], bf16, kind="ExternalOutput")
    fin_out = nc.dram_tensor("fin_out", [128, NB], bf16, kind="ExternalOutput")

    with tile.TileContext(nc) as tc:
        with ExitStack() as ctx:
            cpool = ctx.enter_context(tc.tile_pool(name="const", bufs=1))
            rpool = ctx.enter_context(tc.tile_pool(name="rp", bufs=1))
            spool = ctx.enter_context(tc.tile_pool(name="sp", bufs=2))
            pspool = ctx.enter_context(
                tc.tile_pool(name="ps", bufs=2, space=bass.MemorySpace.PSUM))

            w_t = cpool.tile([128, 128], bf16, tag="w")
            nc.sync.dma_start(w_t[:], wmat[:])

            # window tiles + loads (in order on the sync HWDGE ring)
            R = []
            col = 0
            for wi, ws in enumerate(WIN_STEPS):
                ncols = ws * NB
                rt = rpool.tile([128, ncols], bf16, tag=f"R{wi}", name=f"rt{wi}")
                nc.sync.dma_start(rt[:], x[:, col:col + ncols])
                R.append((rt, col))
                col += ncols

            S, SN = [], []
            for g in range(G):
                st = spool.tile([128, F], bf16, tag=f"S{g}", name=f"st{g}")
                nc.vector.memset(st[:], 1.0)
                sn = cpool.tile([128, F], bf16, tag=f"N{g}")
                S.append(st)
                SN.append(sn)

            # scan
            wi = 0
            wbase = 0
            for s in range(SPAN):
                while s - wbase >= WIN_STEPS[wi]:
                    wbase += WIN_STEPS[wi]
                    wi += 1
                rt, _ = R[wi]
                so = s - wbase
                for g in range(G):
                    ps = pspool.tile([128, F], f32, tag=f"mm{g}")
                    mm = nc.tensor.matmul(ps[:], w_t[:], S[g][:],
                                          start=True, stop=True)
                    if s > 0:
                        mm.ldweights = False
                    sn_new = spool.tile([128, F], bf16, tag=f"S{g}",
                                        name=f"st{g}_{s}")
                    off = so * NB + g * F
                    nc.vector.tensor_mul(sn_new[:], ps[:], rt[:, off:off + F])
                    S[g] = sn_new
                    if s == W - 1:
                        nc.scalar.copy(SN[g][:], S[g][:])

            for g in range(G):
                nc.sync.dma_start(snap_out[:, g * F:(g + 1) * F], SN[g][:])
                nc.sync.dma_start(fin_out[:, g * F:(g + 1) * F], S[g][:])

    nc.compile()
    return nc


def _get_nc():
    if "nc" not in _cache:
        _cache["nc"] = _build()
    return _cache["nc"]


def _log_softmax64(v, axis):
    v = v.astype(np.float64)
    m = v.max(axis=axis, keepdims=True)
    e = np.exp(v - m)
    return v - m - np.log(e.sum(axis=axis, keepdims=True))


def _estimate_delta(log_pdf, T64):
    # E[log c] from a vectorized short scan: 64 parallel probes, 56 steps,
    # burn-in 16 (mixing time is ~6 steps).
    NCH, NST, BURN = 64, 56, 16
    cols = np.arange(NCH) * 997 + 1
    a = np.full((K, NCH), 1.0 / K)
    samples = []
    for s in range(NST):
        p = np.exp(log_pdf[:, cols + s].astype(np.float64))
        a = p * (T64 @ a)
        c = a.sum(axis=0)
        a /= c
        if s >= BURN:
            samples.append(np.log(c))
    return float(np.mean(samples))


def _make_in_maps(log_pdf, T64):
    from ml_dtypes import bfloat16

    Tbf = T64.astype(np.float32).astype(bfloat16)
    delta = _estimate_delta(log_pdf, T64)
    r = Tbf.astype(np.float64).sum(axis=1)
    # host-side emissions: p[k,t] = exp(lp[k,t] - delta - log r_k), bf16
    eb = (-delta - np.log(r)).astype(np.float32)
    P = np.exp(log_pdf + eb[:, None]).astype(bfloat16)

    wm = np.zeros((128, 128), dtype=bfloat16)
    for q in range(4):
        wm[32 * q:32 * q + 32, 32 * q:32 * q + 32] = Tbf.T

    # repack: X[32q+k, s*NB+b] = P[k, c0 + (4b+q)*L + s]
    idx = ((np.arange(NB)[None, :, None] * 4 + np.arange(4)[None, None, :]) * L
           + np.arange(SPAN)[:, None, None])          # [SPAN, NB, 4]
    in_maps = []
    for c in range(NCORES):
        c0 = c * CC * L
        g = P[:, c0:c0 + CC * L + W][:, idx]          # [32, SPAN, NB, 4]
        xc = np.ascontiguousarray(
            g.transpose(3, 0, 1, 2).reshape(128, TOT))
        in_maps.append({"x": xc, "wmat": wm})

    return in_maps, delta


def kernel(log_pdf: np.ndarray, pi: np.ndarray, T: np.ndarray) -> np.ndarray:
    from concourse.bass_utils import run_bass_kernel_spmd

    log_pdf = np.ascontiguousarray(log_pdf, dtype=np.float32)
    log_pi64 = _log_softmax64(pi, 0)
    log_T64 = _log_softmax64(T, 1)
    T64 = np.exp(log_T64)                     # row-stochastic [K, K] f64

    in_maps, delta = _make_in_maps(log_pdf, T64)
    nc = _get_nc()
    res = run_bass_kernel_spmd(nc, in_maps, list(range(NCORES))).results

    # ---- host combine (f64) ----
    LP = log_pdf
    # exact prefix [0, W)
    a = np.exp(log_pi64 + LP[:, 0].astype(np.float64))
    c = a.sum()
    total = np.log(c)
    a /= c
    for t in range(1, W):
        a = np.exp(LP[:, t].astype(np.float64)) * (T64 @ a)
        c = a.sum()
        total += np.log(c)
        a /= c

    # per-chain contributions: log(sum fin) - log(sum snap) + delta*L
    for k in range(NCORES):
        snap = res[k]["snap_out"].astype(np.float64)   # [128, NB]
        fin = res[k]["fin_out"].astype(np.float64)
        ssum = snap.reshape(4, 32, NB).sum(axis=1)     # [4, NB]
        fsum = fin.reshape(4, 32, NB).sum(axis=1)
        total += (np.log(fsum) - np.log(ssum)).sum() + delta * L * CC

    # exact tail [COVERED, N) from the last chain's final state
    fv = res[NCORES - 1]["fin_out"][96:128, NB - 1].astype(np.float64)
    a = fv / fv.sum()
    for t in range(COVERED, N):
        a = np.exp(LP[:, t].astype(np.float64)) * (T64 @ a)
        c = a.sum()
        total += np.log(c)
        a /= c

    return np.float32(total)


# revision 9
# speedup vs baseline: 3.6590x; 1.0281x over previous
"""HMM log-likelihood (log-domain forward algorithm) on 8 Trainium2 cores.

Scaled linear-domain forward algorithm with warmup-halo sequence
parallelism.  N=1e6 timesteps are split into 24960 independent chains
(3120/core); each chain starts from a uniform state W=6 steps before its
owned region of L=40 steps (the HMM mixes with |lambda2|~0.24, so 6
warmup steps reach the bf16 noise floor).  Per core, chains are batched
4-wide across the 128 SBUF partitions (block-diagonal T^T weights on the
PE) with the chain-block index in the matmul free dimension, G=2
interleaved groups of F=390 blocks, so each timestep per group is one
bf16 matmul (T @ S into PSUM) plus one vector multiply by the emission
probabilities.

The emissions exp(log_pdf - delta - log r) are computed on the host in
f32, quantized to bf16, and repacked into the exact per-step SBUF layout
[128, SPAN*NB], so the device does no exp and the DMA is a handful of
large contiguous window loads.  delta = E[log c] makes log|S| a
zero-drift random walk; the bf16 quantization of T factors exactly as
D_r @ T_hat with T_hat row-stochastic, and -log(r) is folded into the
same host-side exponent.  All matmuls share one stationary weight load
(ldweights=False on all but the first per group).  Each chain's
contribution is log(sum(S_final)) - log(sum(S_at_W)) + delta*L,
assembled on the host, which also runs exact f64 scans for the prefix
[0, W) and the short tail.
"""

import sys

for p in ("/opt/trn_rl_repo", "/root/.axon_site", "/root/.axon_site/_ro/trn_rl_repo",
          "/root/.axon_site/_ro/pypackages"):
    if p not in sys.path:
        sys.path.insert(0, p)

import numpy as np

K = 32
N = 1_000_000
NCORES = 8
W = 4             # warmup (halo) steps per chain
L = 40            # owned steps per chain
SPAN = W + L      # 44 sequential steps
CC = 124800 // L  # 3120 chains per core
NB = CC // 4      # 780 four-chain blocks
G = 2             # interleaved compute groups
F = NB // G       # 390 blocks (matmul free dim) per group
TOT = SPAN * NB   # input columns per core
COVERED = W + NCORES * CC * L
NDUMMY = 12       # PE warm-up matmuls (HAM un-throttle) during the ramp

# window sizes in steps (first small for fast ramp)
WIN_STEPS = [2] + [4] * 10 + [2]
assert sum(WIN_STEPS) == SPAN

_cache = {}


def _build():
    import concourse.bass as bass
    import concourse.bacc as bacc
    import concourse.mybir as mybir
    import concourse.tile as tile
    from contextlib import ExitStack

    f32 = mybir.dt.float32
    bf16 = mybir.dt.bfloat16

    nc = bacc.Bacc("TRN2", target_bir_lowering=False, debug=False,
                   num_devices=NCORES)
    x = nc.dram_tensor("x", [128, TOT], bf16, kind="ExternalInput")
    wmat = nc.dram_tensor("wmat", [128, 128], bf16, kind="ExternalInput")
    out = nc.dram_tensor("out", [128, 2 * NB], bf16, kind="ExternalOutput")

    with tile.TileContext(nc) as tc:
        with ExitStack() as ctx:
            cpool = ctx.enter_context(tc.tile_pool(name="const", bufs=1))
            rpool = ctx.enter_context(tc.tile_pool(name="rp", bufs=1))
            spool = ctx.enter_context(tc.tile_pool(name="sp", bufs=2))
            pspool = ctx.enter_context(
                tc.tile_pool(name="ps", bufs=2, space=bass.MemorySpace.PSUM))

            w_t = cpool.tile([128, 128], bf16, tag="w")
            nc.sync.dma_start(w_t[:], wmat[:])

            # PE warm-up burst: garbage matmuls during the DMA ramp keep
            # the HAM activity window busy so the scan runs at 2.4 GHz.
            dw = cpool.tile([128, 128], bf16, tag="dw")
            dx = cpool.tile([128, F], bf16, tag="dx")
            nc.gpsimd.memset(dw[:], 1.0)
            nc.gpsimd.memset(dx[:], 1.0)
            for i in range(NDUMMY):
                dps = pspool.tile([128, F], f32, tag="dmm")
                mmw = nc.tensor.matmul(dps[:], dw[:], dx[:],
                                       start=True, stop=True)
                if i > 0:
                    mmw.ldweights = False

            # window tiles + loads (in order on the sync HWDGE ring)
            R = []
            col = 0
            for wi, ws in enumerate(WIN_STEPS):
                ncols = ws * NB
                rt = rpool.tile([128, ncols], bf16, tag=f"R{wi}", name=f"rt{wi}")
                nc.sync.dma_start(rt[:], x[:, col:col + ncols])
                R.append((rt, col))
                col += ncols

            S, SN = [], []
            for g in range(G):
                st = spool.tile([128, F], bf16, tag=f"S{g}", name=f"st{g}")
                nc.gpsimd.memset(st[:], 1.0)
                sn = cpool.tile([128, F], bf16, tag=f"N{g}")
                S.append(st)
                SN.append(sn)

            # scan
            wi = 0
            wbase = 0
            for s in range(SPAN):
                while s - wbase >= WIN_STEPS[wi]:
                    wbase += WIN_STEPS[wi]
                    wi += 1
                rt, _ = R[wi]
                so = s - wbase
                for g in range(G):
                    ps = pspool.tile([128, F], f32, tag=f"mm{g}")
                    mm = nc.tensor.matmul(ps[:], w_t[:], S[g][:],
                                          start=True, stop=True)
                    if s > 0:
                        mm.ldweights = False
                    sn_new = spool.tile([128, F], bf16, tag=f"S{g}",
                                        name=f"st{g}_{s}")
                    off = so * NB + g * F
                    nc.vector.tensor_mul(sn_new[:], ps[:], rt[:, off:off + F])
                    S[g] = sn_new
                    if s == W - 1:
                        nc.scalar.copy(SN[g][:], S[g][:])

            for g in range(G):
                nc.sync.dma_start(out[:, g * F:(g + 1) * F], SN[g][:])
                nc.sync.dma_start(out[:, NB + g * F:NB + (g + 1) * F], S[g][:])

    nc.compile()
    return nc


def _get_nc():
    if "nc" not in _cache:
        _cache["nc"] = _build()
    return _cache["nc"]


def _log_softmax64(v, axis):
    v = v.astype(np.float64)
    m = v.max(axis=axis, keepdims=True)
    e = np.exp(v - m)
    return v - m - np.log(e.sum(axis=axis, keepdims=True))


def _estimate_delta(log_pdf, T64):
    # E[log c] from a vectorized short scan: 64 parallel probes, 56 steps,
    # burn-in 16 (mixing time is ~6 steps).
    NCH, NST, BURN = 64, 56, 16
    cols = np.arange(NCH) * 997 + 1
    a = np.full((K, NCH), 1.0 / K)
    samples = []
    for s in range(NST):
        p = np.exp(log_pdf[:, cols + s].astype(np.float64))
        a = p * (T64 @ a)
        c = a.sum(axis=0)
        a /= c
        if s >= BURN:
            samples.append(np.log(c))
    return float(np.mean(samples))


def _make_in_maps(log_pdf, T64):
    from ml_dtypes import bfloat16

    Tbf = T64.astype(np.float32).astype(bfloat16)
    delta = _estimate_delta(log_pdf, T64)
    r = Tbf.astype(np.float64).sum(axis=1)
    # host-side emissions: p[k,t] = exp(lp[k,t] - delta - log r_k), bf16
    eb = (-delta - np.log(r)).astype(np.float32)
    P = np.exp(log_pdf + eb[:, None]).astype(bfloat16)

    wm = np.zeros((128, 128), dtype=bfloat16)
    for q in range(4):
        wm[32 * q:32 * q + 32, 32 * q:32 * q + 32] = Tbf.T

    # repack: X[32q+k, s*NB+b] = P[k, c0 + (4b+q)*L + s]
    idx = ((np.arange(NB)[None, :, None] * 4 + np.arange(4)[None, None, :]) * L
           + np.arange(SPAN)[:, None, None])          # [SPAN, NB, 4]
    in_maps = []
    for c in range(NCORES):
        c0 = c * CC * L
        g = P[:, c0:c0 + CC * L + W][:, idx]          # [32, SPAN, NB, 4]
        xc = np.ascontiguousarray(
            g.transpose(3, 0, 1, 2).reshape(128, TOT))
        in_maps.append({"x": xc, "wmat": wm})

    return in_maps, delta


def kernel(log_pdf: np.ndarray, pi: np.ndarray, T: np.ndarray) -> np.ndarray:
    from concourse.bass_utils import run_bass_kernel_spmd

    log_pdf = np.ascontiguousarray(log_pdf, dtype=np.float32)
    log_pi64 = _log_softmax64(pi, 0)
    log_T64 = _log_softmax64(T, 1)
    T64 = np.exp(log_T64)                     # row-stochastic [K, K] f64

    in_maps, delta = _make_in_maps(log_pdf, T64)
    nc = _get_nc()
    res = run_bass_kernel_spmd(nc, in_maps, list(range(NCORES))).results

    # ---- host combine (f64) ----
    LP = log_pdf
    # exact prefix [0, W)
    a = np.exp(log_pi64 + LP[:, 0].astype(np.float64))
    c = a.sum()
    total = np.log(c)
    a /= c
    for t in range(1, W):
        a = np.exp(LP[:, t].astype(np.float64)) * (T64 @ a)
        c = a.sum()
        total += np.log(c)
        a /= c

    # per-chain contributions: log(sum fin) - log(sum snap) + delta*L
    for k in range(NCORES):
        o = res[k]["out"].astype(np.float64)           # [128, 2*NB]
        ssum = o[:, :NB].reshape(4, 32, NB).sum(axis=1)     # [4, NB]
        fsum = o[:, NB:].reshape(4, 32, NB).sum(axis=1)
        total += (np.log(fsum) - np.log(ssum)).sum() + delta * L * CC

    # exact tail [COVERED, N) from the last chain's final state
    fv = res[NCORES - 1]["out"][96:128, 2 * NB - 1].astype(np.float64)
    a = fv / fv.sum()
    for t in range(COVERED, N):
        a = np.exp(LP[:, t].astype(np.float64)) * (T64 @ a)
        c = a.sum()
        total += np.log(c)
        a /= c

    return np.float32(total)
